# revision 25
# baseline (speedup 1.0000x reference)
"""Single-head causal attention (B=4, S=2048, D=1024, dk=128) on 8 TRN2 cores.

Sharding: core c -> batch b=c//2, half h=c%2.
  - h=0 handles query rows [0:512) u [1536:2048), h=1 handles [512:1536)
    (balances causal work: 4+16 vs 8+12 key-tiles per 512-query block).
  - Each core projects the full K/V for its batch (cheaper than an
    intra-pair collective exchange, which measures ~36us on HW).

The host passes activations/weights pre-transposed to [d_model, s] and
pre-cast to bf16 (pure data marshalling; all matmuls/softmax run on
device). Projections contract d_model on the partition dim and emit
qT/kT [dk, s] directly. Scores are computed transposed ([key, query])
so the P@V matmul consumes P tiles as the stationary operand and V in
natural [s, dk] layout; a ones-column appended to V makes the same
matmul accumulate the softmax denominators. The causal mask is applied
as a multiplicative bf16 mask on P, generated on-chip from a per-core
[128, 16] shift table so all 8 cores run one identical program.

K/V are loaded, projected and consumed per 1024-column half so the
attention pipeline overlaps the HBM load phase; P@V accumulates half 0
key tiles in PSUM while half 1 is still loading.
"""

import math

import numpy as np
import ml_dtypes

import concourse.bacc as bacc
import concourse.tile as tile
import concourse.mybir as mybir
from concourse import bass_utils
from concourse.masks import make_identity
from concourse.tile_rust import add_dep_helper

F32 = mybir.dt.float32
BF16 = mybir.dt.bfloat16

B, S, DM, DK = 4, 2048, 1024, 128
NCORES = 8
HALF = S // 2  # query rows per core / key columns per pipeline stage
NCH = DM // 128  # d_model chunks
# program-wide causal shape: query block 0 sees key tiles [0, NJ0),
# block 1 sees [0, NJ1); per-core mask data zeroes what's invalid.
NJ0, NJ1 = 8, 16
VW = DK + 1  # v tiles carry a ones-column for the softmax denominator
SCALE = 1.0 / math.sqrt(DK)
WARMUP_MMS = 24
FILLER_MMS = 48

_CACHE = {}


def _build():
    if "nc" in _CACHE:
        return _CACHE["nc"]
    nc = bacc.Bacc("TRN2", target_bir_lowering=False, debug=False, num_devices=NCORES)

    qx_in = nc.dram_tensor("qx", [DM, HALF], BF16, kind="ExternalInput").ap()
    kx_in = nc.dram_tensor("kx", [DM, S], BF16, kind="ExternalInput").ap()
    vx_in = nc.dram_tensor("vx", [DM, S], BF16, kind="ExternalInput").ap()
    wqT_in = nc.dram_tensor("wqT", [DM, DK], BF16, kind="ExternalInput").ap()
    wkT_in = nc.dram_tensor("wkT", [DM, DK], BF16, kind="ExternalInput").ap()
    wvT_in = nc.dram_tensor("wvT", [DM, DK], BF16, kind="ExternalInput").ap()
    shifts_in = nc.dram_tensor("shifts", [128, 16], F32, kind="ExternalInput").ap()
    out = nc.dram_tensor("out", [HALF, DK], F32, kind="ExternalOutput").ap()

    rings = [nc.sync, nc.scalar, nc.gpsimd]

    with tile.TileContext(nc) as tc:
        with tc.tile_pool(name="const", bufs=1) as const:
            ident = const.tile([128, 128], BF16)
            make_identity(nc, ident)

            # ---- loads: per-(chunk, col-half) DMAs (2-4KB contiguous per
            # partition row), round-robined over 3 issue rings in need order:
            # weights/shifts, qx, then kx/vx half 0, then kx/vx half 1.
            wTs = {}
            for nm, w_dram in (("wq", wqT_in), ("wk", wkT_in), ("wv", wvT_in)):
                wTs[nm] = const.tile([128, NCH, DK], BF16, tag=f"wT_{nm}", name=f"wT{nm}")
            shifts = const.tile([128, 16], F32)

            qx = const.tile([128, NCH, HALF], BF16)
            kxh = [const.tile([128, NCH, HALF], BF16, tag=f"kx{h}", name=f"kxh{h}") for h in range(2)]
            vxh = [const.tile([128, NCH, HALF], BF16, tag=f"vx{h}", name=f"vxh{h}") for h in range(2)]
            qx4 = qx_in.rearrange("(g c p) s -> g p c s", p=128, c=2)
            kx3 = kx_in.rearrange("(c p) s -> c p s", p=128)
            vx3 = vx_in.rearrange("(c p) s -> c p s", p=128)
            ri = 0

            def ld(dst, src):
                nonlocal ri
                rings[ri % 3].dma_start(out=dst, in_=src)
                ri += 1

            ld(wTs["wq"], wqT_in.rearrange("(c p) k -> p c k", p=128))
            ld(shifts, shifts_in)
            for g in range(NCH // 2):
                ld(qx[:, 2 * g : 2 * g + 2, :], qx4[g])
            ld(wTs["wk"], wkT_in.rearrange("(c p) k -> p c k", p=128))
            for c in range(NCH):
                ld(kxh[0][:, c, :], kx3[c][:, 0:HALF])
            for c in range(NCH):
                ld(vxh[0][:, c, :], vx3[c][:, 0:HALF])
            ld(wTs["wv"], wvT_in.rearrange("(c p) k -> p c k", p=128))
            for c in range(NCH):
                ld(kxh[1][:, c, :], kx3[c][:, HALF:S])
            for c in range(NCH):
                ld(vxh[1][:, c, :], vx3[c][:, HALF:S])

            # ---- causal masks built on-chip: mask[p, t, c] = (c >= shift[p, t])
            iota_i = const.tile([128, 512], mybir.dt.int32)
            nc.gpsimd.iota(iota_i, pattern=[[1, 512]], base=0, channel_multiplier=0)
            iota_f = const.tile([128, 512], F32)
            nc.vector.tensor_copy(iota_f, iota_i)
            masks_sb = const.tile([128, 16 * 512], BF16)
            for t in range(16):
                nc.vector.tensor_scalar(
                    masks_sb[:, t * 512 : (t + 1) * 512],
                    iota_f,
                    shifts[:, t : t + 1],
                    None,
                    op0=mybir.AluOpType.is_ge,
                )

            # ---- PE warmup + filler: dummy matmuls keep the HAM clock-gate
            # open while the PE waits for the HBM load phase.
            w_warm = const.tile([128, 512], BF16)
            nc.vector.memset(w_warm, 1.0)
            last_filler = None
            with tc.tile_pool(name="psW", bufs=1, space="PSUM") as psW:
                ps_w = psW.tile([128, 512], F32)
                for _ in range(WARMUP_MMS):
                    nc.tensor.matmul(
                        ps_w[:, 0:128], ident, ident, start=True, stop=True
                    )
                for _ in range(FILLER_MMS):
                    last_filler = nc.tensor.matmul(
                        ps_w, ident, w_warm, start=True, stop=True
                    )

            # ---- persistent projected tensors (split per key half) ----
            qT_sb = const.tile([128, HALF], BF16)
            kTh = [const.tile([128, HALF], BF16, tag=f"kT{h}", name=f"kT{h}") for h in range(2)]
            vTh = [const.tile([128, HALF], BF16, tag=f"vT{h}", name=f"vT{h}") for h in range(2)]
            vsbh = [const.tile([128, NCH, VW], BF16, tag=f"v{h}", name=f"vsb{h}") for h in range(2)]

            with (
                tc.tile_pool(name="psM", bufs=2, space="PSUM") as psM,
                tc.tile_pool(name="psS", bufs=2, space="PSUM") as psS,
                tc.tile_pool(name="psO", bufs=4, space="PSUM") as psO,
                tc.tile_pool(name="pP", bufs=18) as p_pool,
                tc.tile_pool(name="oo", bufs=4) as o_pool,
            ):

                def project(wT, xT, dst, xoff=0):
                    """dst [128, HALF] bf16 += W @ X^T over d chunks."""
                    for blk in range(HALF // 512):
                        acc = psM.tile([128, 512], F32, tag="ps_misc")
                        x0 = xoff + blk * 512
                        for c in range(NCH):
                            mm = nc.tensor.matmul(
                                acc,
                                wT[:, c, :],
                                xT[:, c, x0 : x0 + 512],
                                start=(c == 0),
                                stop=(c == NCH - 1),
                            )
                            if c == 0 and last_filler is not None:
                                add_dep_helper(
                                    mm.ins, last_filler.ins, sync=False,
                                    reason="run filler first",
                                )
                        nc.vector.tensor_copy(dst[:, blk * 512 : (blk + 1) * 512], acc)

                def scores(blk, j_tiles):
                    """score tiles [key, query] -> exp -> optional mask; returns p tiles."""
                    q_cols = slice(blk * 512, (blk + 1) * 512)
                    out_p = []
                    for j in j_tiles:
                        h, jl = j // NCH, j % NCH
                        ps_s = psS.tile([128, 512], F32, tag="score")
                        nc.tensor.matmul(
                            ps_s,
                            kTh[h][:, jl * 128 : (jl + 1) * 128],
                            qT_sb[:, q_cols],
                            start=True,
                            stop=True,
                        )
                        p_t = p_pool.tile([128, 512], BF16, tag="p")
                        nc.scalar.activation(
                            p_t, ps_s, mybir.ActivationFunctionType.Exp, scale=SCALE
                        )
                        if blk == 0 or j >= NJ0:
                            nc.vector.tensor_mul(
                                p_t, p_t, masks_sb[:, j * 512 : (j + 1) * 512]
                            )
                        out_p.append(p_t)
                    return out_p

                def v_natural(h):
                    project(wTs["wv"], vxh[h], vTh[h])
                    for t in range(NCH):
                        ps = psM.tile([128, 128], BF16, tag="ps_misc")
                        nc.tensor.transpose(
                            ps, vTh[h][:, t * 128 : (t + 1) * 128], ident
                        )
                        nc.vector.tensor_copy(vsbh[h][:, t, 0:DK], ps)
                    nc.vector.memset(vsbh[h][:, :, DK : DK + 1], 1.0)

                def div_out(blk, qs, ps_o):
                    rec = o_pool.tile([128, 1], F32, tag="rec")
                    nc.vector.reciprocal(rec, ps_o[:, DK : DK + 1])
                    o_t = o_pool.tile([128, DK], F32, tag="o")
                    nc.vector.tensor_scalar_mul(o_t, ps_o[:, 0:DK], rec)
                    r0 = blk * 512 + qs * 128
                    nc.sync.dma_start(out=out[r0 : r0 + 128, :], in_=o_t)

                # ---------- pipeline ----------
                project(wTs["wq"], qx, qT_sb)

                # half 0: K -> scores (blk0 full, blk1 first half) -> V -> PV
                project(wTs["wk"], kxh[0], kTh[0])
                p_blk0 = scores(0, range(NJ0))
                p_blk1 = scores(1, range(NCH))
                v_natural(0)

                ps_o0 = [psO.tile([128, VW], F32, tag="out", name=f"ps_o0_{i}") for i in range(4)]
                for qs in range(4):
                    for j in range(NJ0):
                        nc.tensor.matmul(
                            ps_o0[qs],
                            p_blk0[j][:, qs * 128 : (qs + 1) * 128],
                            vsbh[0][:, j, :],
                            start=(j == 0),
                            stop=(j == NJ0 - 1),
                        )
                    div_out(0, qs, ps_o0[qs])

                ps_o1 = [psO.tile([128, VW], F32, tag="out", name=f"ps_o1_{i}") for i in range(4)]
                for qs in range(4):
                    for j in range(NCH):
                        nc.tensor.matmul(
                            ps_o1[qs],
                            p_blk1[j][:, qs * 128 : (qs + 1) * 128],
                            vsbh[0][:, j, :],
                            start=(j == 0),
                            stop=False,
                        )

                # half 1: K -> scores (blk1 second half) -> V -> finish PV
                project(wTs["wk"], kxh[1], kTh[1])
                p_blk1b = scores(1, range(NCH, NJ1))
                v_natural(1)
                for qs in range(4):
                    for jl in range(NCH):
                        nc.tensor.matmul(
                            ps_o1[qs],
                            p_blk1b[jl][:, qs * 128 : (qs + 1) * 128],
                            vsbh[1][:, jl, :],
                            start=False,
                            stop=(jl == NCH - 1),
                        )
                    div_out(1, qs, ps_o1[qs])

    nc.compile()
    _CACHE["nc"] = nc
    return nc


def _shift_block(h):
    """[128, 16] f32: mask[p, t, c] = (c >= shift) == (key 128t+p <= query qb+c)."""
    qbase = (0, 1536) if h == 0 else (512, 1024)
    p = np.arange(128, dtype=np.float32)[:, None]
    t = np.arange(16, dtype=np.float32)[None, :]
    qb = np.where(t < NJ0, qbase[0], qbase[1])
    return (128.0 * t + p - qb).astype(np.float32)


def kernel(**inputs):
    queries = np.asarray(inputs["queries"], dtype=np.float32)
    keys = np.asarray(inputs["keys"], dtype=np.float32)
    values = np.asarray(inputs["values"], dtype=np.float32)

    nc = _build()
    bf = ml_dtypes.bfloat16
    shifts = [_shift_block(0), _shift_block(1)]
    qrows = [np.r_[0:512, 1536:2048], np.r_[512:1536]]
    wT = {
        nm: np.ascontiguousarray(np.asarray(inputs[nm], dtype=np.float32).T).astype(bf)
        for nm in ("Wq", "Wk", "Wv")
    }
    kxs = [np.ascontiguousarray(keys[b].T).astype(bf) for b in range(B)]
    vxs = [np.ascontiguousarray(values[b].T).astype(bf) for b in range(B)]

    in_maps = []
    for c in range(NCORES):
        b, h = c // 2, c % 2
        in_maps.append(
            {
                "qx": np.ascontiguousarray(queries[b][qrows[h]].T).astype(bf),
                "kx": kxs[b],
                "vx": vxs[b],
                "wqT": wT["Wq"],
                "wkT": wT["Wk"],
                "wvT": wT["Wv"],
                "shifts": shifts[h],
            }
        )

    res = bass_utils.run_bass_kernel_spmd(
        nc, in_maps, list(range(NCORES)), **_CACHE.get("run_kwargs", {})
    )
    _CACHE["last_result"] = res

    out = np.empty((B, S, DK), dtype=np.float32)
    for c in range(NCORES):
        b, h = c // 2, c % 2
        out[b][qrows[h]] = res.results[c]["out"]
    return out


# revision 26
# speedup vs baseline: 1.1169x; 1.1169x over previous
"""Single-head causal attention (B=4, S=2048, D=1024, dk=128) on 8 TRN2 cores.

Sharding: core c -> batch b=c//2, half h=c%2.
  - h=0 handles query rows [0:512) u [1536:2048), h=1 handles [512:1536)
    (balances causal work: 4+16 vs 8+12 key-tiles per 512-query block).
  - Each core projects the full K/V for its batch (cheaper than an
    intra-pair collective exchange, which measures ~36us on HW).

The host passes activations/weights pre-transposed to [d_model, s] and
pre-cast to bf16 (pure data marshalling; all matmuls/softmax run on
device). Projections contract d_model on the partition dim and emit
qT/kT [dk, s] directly. Scores are computed transposed ([key, query])
so the P@V matmul consumes P tiles as the stationary operand and V in
natural [s, dk] layout; a ones-column appended to V makes the same
matmul accumulate the softmax denominators. The causal mask is applied
as a multiplicative bf16 mask on P, generated on-chip from a per-core
[128, 16] shift table so all 8 cores run one identical program.

K/V are loaded, projected and consumed per 1024-column half so the
attention pipeline overlaps the HBM load phase; P@V accumulates half 0
key tiles in PSUM while half 1 is still loading.
"""

import math

import numpy as np
import ml_dtypes

import concourse.bacc as bacc
import concourse.tile as tile
import concourse.mybir as mybir
from concourse import bass_utils
from concourse.masks import make_identity
from concourse.tile_rust import add_dep_helper

F32 = mybir.dt.float32
BF16 = mybir.dt.bfloat16

B, S, DM, DK = 4, 2048, 1024, 128
NCORES = 8
HALF = S // 2  # query rows per core / key columns per pipeline stage
NCH = DM // 128  # d_model chunks
# program-wide causal shape: query block 0 sees key tiles [0, NJ0),
# block 1 sees [0, NJ1); per-core mask data zeroes what's invalid.
NJ0, NJ1 = 8, 16
VW = DK + 1  # v tiles carry a ones-column for the softmax denominator
SCALE = 1.0 / math.sqrt(DK)
WARMUP_MMS = 24
FILLER_MMS = 40

_CACHE = {}


def _build():
    if "nc" in _CACHE:
        return _CACHE["nc"]
    nc = bacc.Bacc("TRN2", target_bir_lowering=False, debug=False, num_devices=NCORES)

    qx_in = nc.dram_tensor("qx", [DM, HALF], BF16, kind="ExternalInput").ap()
    kx_in = nc.dram_tensor("kx", [DM, S], BF16, kind="ExternalInput").ap()
    vx_in = nc.dram_tensor("vx", [DM, S], BF16, kind="ExternalInput").ap()
    wqT_in = nc.dram_tensor("wqT", [DM, DK], BF16, kind="ExternalInput").ap()
    wkT_in = nc.dram_tensor("wkT", [DM, DK], BF16, kind="ExternalInput").ap()
    wvT_in = nc.dram_tensor("wvT", [DM, DK], BF16, kind="ExternalInput").ap()
    shifts_in = nc.dram_tensor("shifts", [128, 16], F32, kind="ExternalInput").ap()
    out = nc.dram_tensor("out", [HALF, DK], F32, kind="ExternalOutput").ap()

    rings = [nc.sync, nc.scalar, nc.gpsimd]

    with tile.TileContext(nc) as tc:
        with tc.tile_pool(name="const", bufs=1) as const:
            ident = const.tile([128, 128], BF16)
            make_identity(nc, ident)

            # ---- loads: per-(chunk, col-half) DMAs (2-4KB contiguous per
            # partition row), round-robined over 3 issue rings in need order:
            # weights/shifts, qx, then kx/vx half 0, then kx/vx half 1.
            wTs = {}
            for nm, w_dram in (("wq", wqT_in), ("wk", wkT_in), ("wv", wvT_in)):
                wTs[nm] = const.tile([128, NCH, DK], BF16, tag=f"wT_{nm}", name=f"wT{nm}")
            shifts = const.tile([128, 16], F32)

            qx = const.tile([128, NCH, HALF], BF16)
            kx = const.tile([128, NCH, S], BF16)
            vxh = [const.tile([128, NCH, HALF], BF16, tag=f"vx{h}", name=f"vxh{h}") for h in range(2)]
            qx3 = qx_in.rearrange("(c p) s -> c p s", p=128)
            kx3 = kx_in.rearrange("(c p) s -> c p s", p=128)
            vx3 = vx_in.rearrange("(c p) s -> c p s", p=128)
            ri = 0

            def ld(dst, src):
                nonlocal ri
                rings[ri % 3].dma_start(out=dst, in_=src)
                ri += 1

            ld(wTs["wq"], wqT_in.rearrange("(c p) k -> p c k", p=128))
            ld(shifts, shifts_in)
            for c in range(NCH):
                ld(qx[:, c, :], qx3[c])
            ld(wTs["wk"], wkT_in.rearrange("(c p) k -> p c k", p=128))
            for c in range(NCH):
                ld(kx[:, c, :], kx3[c])
            for c in range(NCH):
                ld(vxh[0][:, c, :], vx3[c][:, 0:HALF])
            ld(wTs["wv"], wvT_in.rearrange("(c p) k -> p c k", p=128))
            for c in range(NCH):
                ld(vxh[1][:, c, :], vx3[c][:, HALF:S])

            # ---- causal masks built on-chip: mask[p, t, c] = (c >= shift[p, t])
            iota_i = const.tile([128, 512], mybir.dt.int32)
            nc.gpsimd.iota(iota_i, pattern=[[1, 512]], base=0, channel_multiplier=0)
            iota_f = const.tile([128, 512], F32)
            nc.vector.tensor_copy(iota_f, iota_i)
            masks_sb = const.tile([128, 16 * 512], BF16)
            for t in range(16):
                nc.vector.tensor_scalar(
                    masks_sb[:, t * 512 : (t + 1) * 512],
                    iota_f,
                    shifts[:, t : t + 1],
                    None,
                    op0=mybir.AluOpType.is_ge,
                )

            # ---- PE warmup + filler: dummy matmuls keep the HAM clock-gate
            # open while the PE waits for the HBM load phase.
            w_warm = const.tile([128, 512], BF16)
            nc.vector.memset(w_warm, 1.0)
            last_filler = None
            with tc.tile_pool(name="psW", bufs=1, space="PSUM") as psW:
                ps_w = psW.tile([128, 512], F32)
                for _ in range(WARMUP_MMS):
                    nc.tensor.matmul(
                        ps_w[:, 0:128], ident, ident, start=True, stop=True
                    )
                for _ in range(FILLER_MMS):
                    last_filler = nc.tensor.matmul(
                        ps_w, ident, w_warm, start=True, stop=True
                    )

            # ---- persistent projected tensors (split per key half) ----
            qT_sb = const.tile([128, HALF], BF16)
            kTh = [const.tile([128, HALF], BF16, tag=f"kT{h}", name=f"kT{h}") for h in range(2)]
            vTh = [const.tile([128, HALF], BF16, tag=f"vT{h}", name=f"vT{h}") for h in range(2)]
            vsbh = [const.tile([128, NCH, VW], BF16, tag=f"v{h}", name=f"vsb{h}") for h in range(2)]

            with (
                tc.tile_pool(name="psM", bufs=2, space="PSUM") as psM,
                tc.tile_pool(name="psS", bufs=2, space="PSUM") as psS,
                tc.tile_pool(name="psO", bufs=4, space="PSUM") as psO,
                tc.tile_pool(name="pP", bufs=18) as p_pool,
                tc.tile_pool(name="oo", bufs=4) as o_pool,
            ):

                def project(wT, xT, dst, xoff=0):
                    """dst [128, HALF] bf16 += W @ X^T over d chunks."""
                    for blk in range(HALF // 512):
                        acc = psM.tile([128, 512], F32, tag="ps_misc")
                        x0 = xoff + blk * 512
                        for c in range(NCH):
                            mm = nc.tensor.matmul(
                                acc,
                                wT[:, c, :],
                                xT[:, c, x0 : x0 + 512],
                                start=(c == 0),
                                stop=(c == NCH - 1),
                            )
                            if c == 0 and last_filler is not None:
                                add_dep_helper(
                                    mm.ins, last_filler.ins, sync=False,
                                    reason="run filler first",
                                )
                        nc.vector.tensor_copy(dst[:, blk * 512 : (blk + 1) * 512], acc)

                def scores(blk, j_tiles):
                    """score tiles [key, query] -> exp -> optional mask; returns p tiles."""
                    q_cols = slice(blk * 512, (blk + 1) * 512)
                    out_p = []
                    for j in j_tiles:
                        h, jl = j // NCH, j % NCH
                        ps_s = psS.tile([128, 512], F32, tag="score")
                        nc.tensor.matmul(
                            ps_s,
                            kTh[h][:, jl * 128 : (jl + 1) * 128],
                            qT_sb[:, q_cols],
                            start=True,
                            stop=True,
                        )
                        p_t = p_pool.tile([128, 512], BF16, tag="p")
                        nc.scalar.activation(
                            p_t, ps_s, mybir.ActivationFunctionType.Exp, scale=SCALE
                        )
                        if blk == 0 or j >= NJ0:
                            nc.vector.tensor_mul(
                                p_t, p_t, masks_sb[:, j * 512 : (j + 1) * 512]
                            )
                        out_p.append(p_t)
                    return out_p

                def v_natural(h):
                    project(wTs["wv"], vxh[h], vTh[h])
                    for t in range(NCH):
                        ps = psM.tile([128, 128], BF16, tag="ps_misc")
                        nc.tensor.transpose(
                            ps, vTh[h][:, t * 128 : (t + 1) * 128], ident
                        )
                        nc.vector.tensor_copy(vsbh[h][:, t, 0:DK], ps)
                    nc.vector.memset(vsbh[h][:, :, DK : DK + 1], 1.0)

                def div_out(blk, qs, ps_o):
                    rec = o_pool.tile([128, 1], F32, tag="rec")
                    nc.vector.reciprocal(rec, ps_o[:, DK : DK + 1])
                    o_t = o_pool.tile([128, DK], F32, tag="o")
                    nc.vector.tensor_scalar_mul(o_t, ps_o[:, 0:DK], rec)
                    r0 = blk * 512 + qs * 128
                    nc.sync.dma_start(out=out[r0 : r0 + 128, :], in_=o_t)

                # ---------- pipeline ----------
                project(wTs["wq"], qx, qT_sb)

                # half 0: K -> scores (blk0 full, blk1 first half) -> V -> PV
                project(wTs["wk"], kx, kTh[0], xoff=0)
                p_blk0 = scores(0, range(NJ0))
                p_blk1 = scores(1, range(NCH))
                v_natural(0)

                ps_o0 = [psO.tile([128, VW], F32, tag="out", name=f"ps_o0_{i}") for i in range(4)]
                for qs in range(4):
                    for j in range(NJ0):
                        nc.tensor.matmul(
                            ps_o0[qs],
                            p_blk0[j][:, qs * 128 : (qs + 1) * 128],
                            vsbh[0][:, j, :],
                            start=(j == 0),
                            stop=(j == NJ0 - 1),
                        )
                    div_out(0, qs, ps_o0[qs])

                ps_o1 = [psO.tile([128, VW], F32, tag="out", name=f"ps_o1_{i}") for i in range(4)]
                for qs in range(4):
                    for j in range(NCH):
                        nc.tensor.matmul(
                            ps_o1[qs],
                            p_blk1[j][:, qs * 128 : (qs + 1) * 128],
                            vsbh[0][:, j, :],
                            start=(j == 0),
                            stop=False,
                        )

                # half 1: K -> scores (blk1 second half) -> V -> finish PV
                project(wTs["wk"], kx, kTh[1], xoff=HALF)
                p_blk1b = scores(1, range(NCH, NJ1))
                v_natural(1)
                for qs in range(4):
                    for jl in range(NCH):
                        nc.tensor.matmul(
                            ps_o1[qs],
                            p_blk1b[jl][:, qs * 128 : (qs + 1) * 128],
                            vsbh[1][:, jl, :],
                            start=False,
                            stop=(jl == NCH - 1),
                        )
                    div_out(1, qs, ps_o1[qs])

    nc.compile()
    _CACHE["nc"] = nc
    return nc


def _shift_block(h):
    """[128, 16] f32: mask[p, t, c] = (c >= shift) == (key 128t+p <= query qb+c)."""
    qbase = (0, 1536) if h == 0 else (512, 1024)
    p = np.arange(128, dtype=np.float32)[:, None]
    t = np.arange(16, dtype=np.float32)[None, :]
    qb = np.where(t < NJ0, qbase[0], qbase[1])
    return (128.0 * t + p - qb).astype(np.float32)


def kernel(**inputs):
    queries = np.asarray(inputs["queries"], dtype=np.float32)
    keys = np.asarray(inputs["keys"], dtype=np.float32)
    values = np.asarray(inputs["values"], dtype=np.float32)

    nc = _build()
    bf = ml_dtypes.bfloat16
    shifts = [_shift_block(0), _shift_block(1)]
    qrows = [np.r_[0:512, 1536:2048], np.r_[512:1536]]
    wT = {
        nm: np.ascontiguousarray(np.asarray(inputs[nm], dtype=np.float32).T).astype(bf)
        for nm in ("Wq", "Wk", "Wv")
    }
    kxs = [np.ascontiguousarray(keys[b].T).astype(bf) for b in range(B)]
    vxs = [np.ascontiguousarray(values[b].T).astype(bf) for b in range(B)]

    in_maps = []
    for c in range(NCORES):
        b, h = c // 2, c % 2
        in_maps.append(
            {
                "qx": np.ascontiguousarray(queries[b][qrows[h]].T).astype(bf),
                "kx": kxs[b],
                "vx": vxs[b],
                "wqT": wT["Wq"],
                "wkT": wT["Wk"],
                "wvT": wT["Wv"],
                "shifts": shifts[h],
            }
        )

    res = bass_utils.run_bass_kernel_spmd(
        nc, in_maps, list(range(NCORES)), **_CACHE.get("run_kwargs", {})
    )
    _CACHE["last_result"] = res

    out = np.empty((B, S, DK), dtype=np.float32)
    for c in range(NCORES):
        b, h = c // 2, c % 2
        out[b][qrows[h]] = res.results[c]["out"]
    return out
